# revision 1
# baseline (speedup 1.0000x reference)
"""Causal self-attention on 8 Trainium2 NeuronCores.

Sharding: core c = 4*b + g handles batch b (of 2) and head-group g (4 of 16
heads).  Weights are column-sliced (Wq/Wk/Wv) / row-sliced (Wp) per head
group; each core returns a partial output [T, C] in bf16 that the host
upcasts, sums per batch, and biases.

Per-core dataflow (all matmul operands bf16, fp32 PSUM accumulation):
  x^T loaded as [128, kt, t] chunk tiles.
  Q^T/K^T = W^T-slice @ x^T -> [128 d, 512 t] per (j, chunk); the per-d bias
    is folded into the PSUM->SBUF eviction (DVE tensor_scalar_add).
  V = x @ Wv_aug -> [t 128, 260] per t-block (aug: 65 cols/head, every 65th
    col is reset to 1.0 by a strided memset -> P@V also yields the softmax
    denominator in column 64 of each head's output).
  S^T[k,q] = K^T.T @ Q^T per (head-pair, k-block): two 64-contraction
    matmuls into one [128, 2, 512] PSUM tile, one merged exp (ACT) for both
    heads, diagonal blocks masked by a Pool multiply.
  PV reoriented [q, d]: y[q, 0:65] += P^T-slice.T @ V-slice per k-block
    (K=128, M=128, N=65 -- full PE array vs the [d, q] orientation's K=64).
    Softmax division is a per-partition reciprocal + tensor_scalar.
  y tiles are PE-transposed back to [d, q] (bv folded into the PSUM->SBUF
    eviction of the transpose), then out_partial = y^T.T @ Wp staged bf16
    and DMA'd per t-block.
"""

import re
import sys

sys.path.insert(0, "/opt/trn_rl_repo")

import numpy as np
import ml_dtypes

import bass_rust
import concourse.bass as bass
import concourse.mybir as mybir
from concourse.tile import TileContext
from concourse.vector_clock import ScopedClock

BF16 = ml_dtypes.bfloat16

T = 2048          # sequence length per batch
C = 1024          # model dim
DHG = 256         # head dims per core (4 heads x 64)
DH = 64           # head dim
NH = 4            # heads per core
DVA = NH * (DH + 1)  # 260: V augmented with a ones-column per head
KT = C // 128     # 8 full contraction tiles
TB = T // 128     # 16 row tiles
QC = T // 512     # 4 query chunks
SCALE = 0.125     # 1/sqrt(64)


class SplitDrainTileContext(TileContext):
    """Walrus TRN2 codegen rejects >4 sync waits on one instruction; the
    stock TileContext exit drain carries one wait per live proc.  Split them
    into single-wait drains chained on the sync sequencer."""

    def _drain_and_barrier(self, tick_clock, wait_clock):
        gc = tick_clock.global_clock
        ticks = [int(x) for x in re.findall(r"\d+", repr(gc))]
        for proc, tick in [(i, t) for i, t in enumerate(ticks) if t > 0]:
            sub = bass_rust.VectorClock()
            sub.require_at_least(proc, tick)
            inst = self.nc.sync.drain()
            wait_clock.add_sem_waits(inst.ins, ScopedClock({None: sub}))
        self.nc.sync.drain()
        self.nc.all_engine_barrier()
        assert self.sems is not None
        popped = self.nc._tile_sem_poison_stack.pop()
        assert popped is self._sem_poison
        self.nc.clear_and_free_semaphores(list(self.sems.allocated().values()))
        self.nc.all_engine_barrier()


def _split_excess_waits(nc, max_waits=1):
    """Walrus TRN2 codegen allows only ~2 sync waits per instruction.
    Hoist any excess onto same-engine InstNoOp carriers placed immediately
    before the instruction — the engine is in-order, so semantics are
    identical."""
    ctr = 0
    for fn in nc.m.functions:
        for bb in fn.blocks:
            new = []
            for inst in bb.instructions:
                si = inst.sync_info
                if (si and si.on_wait and len(si.on_wait) > max_waits
                        and "Unassigned" not in str(inst.engine)):
                    waits = list(si.on_wait)
                    for w in waits[:-max_waits]:
                        ctr += 1
                        nop = bass_rust.InstNoOp(
                            name=f"wsplit-{ctr}", ins=[], outs=[])
                        nop.engine = inst.engine
                        nop.sync_info = bass_rust.SyncInfo(
                            on_wait=[w], on_update=[])
                        new.append(nop)
                    si.on_wait = waits[-max_waits:]
                new.append(inst)
            bb.instructions = new


def build_attention_nc(legalize=True):
    nc = bass.Bass(num_devices=8)
    dt = mybir.dt

    xt = nc.dram_tensor("xt", [C, T], dt.bfloat16, kind="ExternalInput")
    wq = nc.dram_tensor("wq", [C, DHG], dt.bfloat16, kind="ExternalInput")
    wk = nc.dram_tensor("wk", [C, DHG], dt.bfloat16, kind="ExternalInput")
    wv = nc.dram_tensor("wv", [C, DVA], dt.bfloat16, kind="ExternalInput")
    wp = nc.dram_tensor("wp", [DHG, C], dt.bfloat16, kind="ExternalInput")
    # mask (x2, for the two heads of a pair) | 128x128 identity
    mi = nc.dram_tensor("mi", [128, 3 * 128], dt.bfloat16, kind="ExternalInput")
    # per-partition bias columns: bq(j0) bq(j1) bk(j0) bk(j1) bv(j0) bv(j1)
    bias = nc.dram_tensor("bias", [128, 6], dt.float32, kind="ExternalInput")
    out = nc.dram_tensor("out", [T, C], dt.bfloat16, kind="ExternalOutput")

    xt_r = xt.rearrange("(k p) t -> p k t", p=128)
    wq_r = wq.rearrange("(k p) d -> p k d", p=128)
    wk_r = wk.rearrange("(k p) d -> p k d", p=128)
    wv_r = wv.rearrange("(k p) d -> p k d", p=128)

    with SplitDrainTileContext(nc) as tc:
        with (
            tc.tile_pool(name="weights", bufs=1) as wpool,
            tc.tile_pool(name="acts", bufs=1) as apool,
            tc.tile_pool(name="ptiles", bufs=38) as ppool,
            tc.tile_pool(name="small", bufs=4) as spool,
            tc.tile_pool(name="ostage", bufs=3) as opool,
            tc.tile_pool(name="ps_mm", bufs=2, space="PSUM") as ps_mm,
            tc.tile_pool(name="ps_s", bufs=2, space="PSUM") as ps_s,
            tc.tile_pool(name="ps_pvt", bufs=2, space="PSUM") as ps_pvt,
        ):
            # ---- input loads: wq + xt chunk 0 first (both halved so the
            # first Q chain starts after 2 DMAs), then K/V weights, the
            # remaining x^T chunks, and Wp. -------------------------------
            mi_sb = wpool.tile([128, 3, 128], dt.bfloat16)
            bias_sb = wpool.tile([128, 6], dt.float32)
            wq_sb = [wpool.tile([128, 4, DHG], dt.bfloat16, name=f"wq{h}")
                     for h in range(2)]
            wk_sb = [wpool.tile([128, 4, DHG], dt.bfloat16, name=f"wk{h}")
                     for h in range(2)]
            wv_sb = [wpool.tile([128, 4, DVA], dt.bfloat16, name=f"wv{h}")
                     for h in range(2)]
            xt_c0 = [apool.tile([128, 4, 512], dt.bfloat16, name=f"xtc0_{h}")
                     for h in range(2)]
            xt_c = [apool.tile([128, KT, 512], dt.bfloat16, name=f"xtc{cc}")
                    for cc in range(1, QC)]
            wp_sb = wpool.tile([128, 2, C], dt.bfloat16)

            nc.sync.dma_start(xt_c0[0][:], xt_r[:, 0:4, 0:512])
            nc.sync.dma_start(wq_sb[0][:], wq_r[:, 0:4, :])
            nc.sync.dma_start(xt_c0[1][:], xt_r[:, 4:8, 0:512])
            nc.sync.dma_start(wq_sb[1][:], wq_r[:, 4:8, :])
            nc.sync.dma_start(mi_sb[:], mi.rearrange("p (g f) -> p g f", f=128))
            nc.sync.dma_start(bias_sb[:], bias[:])
            nc.sync.dma_start(wk_sb[0][:], wk_r[:, 0:4, :])
            nc.sync.dma_start(wk_sb[1][:], wk_r[:, 4:8, :])
            nc.sync.dma_start(wv_sb[0][:], wv_r[:, 0:4, :])
            nc.sync.dma_start(wv_sb[1][:], wv_r[:, 4:8, :])
            for cc in range(1, QC):
                nc.sync.dma_start(
                    xt_c[cc - 1][:], xt_r[:, :, cc * 512:(cc + 1) * 512])
            nc.sync.dma_start(wp_sb[:], wp.rearrange("(k p) d -> p k d", p=128))

            def xt_at(c, kt):
                if c == 0:
                    return xt_c0[kt // 4][:, kt % 4, :]
                return xt_c[c - 1][:, kt, :]

            def wqk_at(w_sb, kt):
                return w_sb[kt // 4][:, kt % 4, :]

            # SBUF activation tiles
            qt_sb = [[apool.tile([128, 512], dt.bfloat16, name=f"qt{j}_{c}")
                      for c in range(QC)] for j in range(2)]
            kt_sb = [[apool.tile([128, 512], dt.bfloat16, name=f"kt{j}_{c}")
                      for c in range(QC)] for j in range(2)]
            v_sb = [apool.tile([128, DVA], dt.bfloat16, name=f"v{tb}")
                    for tb in range(TB)]
            yt_sb = [[apool.tile([128, 512], dt.bfloat16, name=f"yt{j}_{c}")
                      for c in range(QC)] for j in range(2)]

            # ---- unit emitters ------------------------------------------
            def qk_chain(w_sb, dst, bcol, j, c):
                ps = ps_mm.tile([128, 512], dt.float32, tag="mm")
                for kt in range(KT):
                    nc.tensor.matmul(
                        ps[:],
                        wqk_at(w_sb, kt)[:, j * 128:(j + 1) * 128],
                        xt_at(c, kt),
                        start=(kt == 0), stop=(kt == KT - 1),
                    )
                nc.vector.tensor_scalar_add(
                    dst[j][c][:], ps[:], bias_sb[:, bcol:bcol + 1])

            def qk_half(w_sb, dst, bcol, j, c, half):
                """Self-contained half-width projection: full 8-ktile
                accumulation over 256 of the 512 t-columns + eviction, so
                arbitrary units may interleave without breaking a group."""
                ps = ps_mm.tile([128, 256], dt.float32, tag="mm",
                                name=f"qk{bcol}_{c}_{half}")
                t0 = half * 256
                for kt in range(KT):
                    nc.tensor.matmul(
                        ps[:],
                        wqk_at(w_sb, kt)[:, j * 128:(j + 1) * 128],
                        xt_at(c, kt)[:, t0:t0 + 256],
                        start=(kt == 0), stop=(kt == KT - 1),
                    )
                nc.vector.tensor_scalar_add(
                    dst[j][c][:, t0:t0 + 256], ps[:], bias_sb[:, bcol:bcol + 1])

            def v_half(tb, half):
                ps = ps_mm.tile([128, 130], dt.float32, tag="mm",
                                name=f"v{tb}_{half}")
                d0 = half * 130
                for kt in range(KT):
                    nc.tensor.matmul(
                        ps[:],
                        xt_at(tb // 4, kt)[:, (tb % 4) * 128:(tb % 4 + 1) * 128],
                        wv_sb[kt // 4][:, kt % 4, d0:d0 + 130],
                        start=(kt == 0), stop=(kt == KT - 1),
                    )
                nc.vector.tensor_copy(v_sb[tb][:, d0:d0 + 130], ps[:])
                # ones-columns for the softmax denominators
                nc.vector.memset(v_sb[tb][:, d0 + DH:d0 + 130:DH + 1], 1.0)

            p_tiles = {}   # (hb, c) -> list over jj of [128, 2, 512] tiles

            def s_pair(hb, c, jj):
                qq0 = max(0, (jj - 4 * c) * 128)
                pss = ps_s.tile([128, 2, 512], dt.float32, tag="s")
                for he in range(2):
                    hp = he * 64
                    nc.tensor.matmul(
                        pss[:, he, qq0:512],
                        kt_sb[hb][jj // 4][hp:hp + 64,
                                           (jj % 4) * 128:(jj % 4 + 1) * 128],
                        qt_sb[hb][c][hp:hp + 64, qq0:512],
                        start=True, stop=True,
                    )
                p = ppool.tile([128, 2, 512], dt.bfloat16, tag="p")
                nc.scalar.activation(
                    p[:, :, qq0:512], pss[:, :, qq0:512],
                    mybir.ActivationFunctionType.Exp, scale=SCALE,
                )
                if jj >= 4 * c:
                    nc.gpsimd.tensor_tensor(
                        p[:, :, qq0:qq0 + 128], p[:, :, qq0:qq0 + 128],
                        mi_sb[:, 0:2, :], mybir.AluOpType.mult,
                    )
                p_tiles[(hb, c)].append(p)

            y_cur = {}     # j -> y_sb tile for the q-block being reduced

            def pv_head(qb, h):
                c, qo = qb // 4, qb % 4
                hb, he = h // 2, h % 2
                ps = ps_pvt.tile([128, 512], dt.float32, tag="pvt",
                                 name=f"pv{qb}_{h}")
                for jj in range(qb + 1):
                    nc.tensor.matmul(
                        ps[:, 0:DH + 1],
                        p_tiles[(hb, c)][jj][:, he, qo * 128:(qo + 1) * 128],
                        v_sb[jj][:, h * 65:h * 65 + 65],
                        start=(jj == 0), stop=(jj == qb),
                    )
                r = spool.tile([128, 1], dt.float32, tag="r", name=f"r{qb}_{h}")
                nc.vector.reciprocal(r[:], ps[:, DH:DH + 1])
                if he == 0:
                    y_cur[(qb, hb)] = spool.tile(
                        [128, 128], dt.bfloat16, tag="y", bufs=8,
                        name=f"y{qb}_{hb}")
                nc.vector.tensor_scalar_mul(
                    y_cur[(qb, hb)][:, he * 64:(he + 1) * 64], ps[:, 0:DH], r[:])

            y_t = {}       # qb -> (y_sb j0, y_sb j1) awaiting transpose

            def transpose_j(qb, j):
                c, qo = qb // 4, qb % 4
                pst = ps_pvt.tile([128, 128], dt.bfloat16, tag="pvt",
                                  name=f"tp{qb}_{j}")
                nc.tensor.transpose(pst[:], y_t[qb][j][:], mi_sb[:, 2, :])
                nc.vector.tensor_scalar_add(
                    yt_sb[j][c][:, qo * 128:(qo + 1) * 128], pst[:],
                    bias_sb[:, 4 + j:5 + j])

            def op_half(tb, e, pool=None):
                pool = pool or ps_mm
                ps = pool.tile([128, 512], dt.float32, tag=pool.name[3:],
                               name=f"op{tb}_{e}")
                for j in range(2):
                    nc.tensor.matmul(
                        ps[:],
                        yt_sb[j][tb // 4][:, (tb % 4) * 128:(tb % 4 + 1) * 128],
                        wp_sb[:, j, e * 512:(e + 1) * 512],
                        start=(j == 0), stop=(j == 1),
                    )
                st = opool.tile([128, 512], dt.bfloat16, tag="st",
                                name=f"st{tb}_{e}")
                # early-draining out-projs overlap the mid-kernel where ACT
                # has slack; keep the endgame (tb>=10) copies off ACT, which
                # is saturated by the last chunk's exps there
                if e == 1 and tb < 10:
                    nc.scalar.copy(st[:], ps[:])
                else:
                    nc.vector.tensor_copy(st[:], ps[:])
                nc.sync.dma_start(
                    out[tb * 128:(tb + 1) * 128, e * 512:(e + 1) * 512],
                    st[:])

            # ---- emission schedule --------------------------------------
            # PE warmup: ~3us of dummy matmuls ramps the PE p-state to full
            # speed while the first DMAs are still in flight.
            warm_sb = wpool.tile([128, 128], dt.bfloat16)
            nc.vector.memset(warm_sb[:], 0.0)
            psw = ps_mm.tile([128, 128], dt.float32, tag="mm", name="warm")
            NWARM = 50
            for i in range(NWARM):
                nc.tensor.matmul(psw[:], warm_sb[:], warm_sb[:],
                                 start=(i == 0), stop=(i == NWARM - 1))

            # chunk 0's Q/K projections
            qk_chain(wq_sb, qt_sb, 0, 0, 0)
            qk_chain(wq_sb, qt_sb, 1, 1, 0)
            qk_chain(wk_sb, kt_sb, 2, 0, 0)
            qk_chain(wk_sb, kt_sb, 3, 1, 0)
            for hb in range(2):
                for c in range(QC):
                    p_tiles[(hb, c)] = []

            # Global flattened S stream: all (c, jj, hb) in order, zipped
            # with a single filler queue drained proportionally (plus
            # stall-covering) so PE stays fed and ACT (exp) never gates a
            # bunched region.  PE-time estimates pace the zip; correctness
            # only relies on emission order, the tile deps do the rest.
            s_units = [(c, jj, hb)
                       for c in range(QC)
                       for jj in range(4 * c + 4)
                       for hb in range(2)]
            NS = len(s_units)

            PE_CYC = 0.4166
            fillq = []        # (cost_ns, closure, kind, key)

            def push(cost, fn, kind=None, key=None, front=False):
                if front:
                    fillq.insert(0, (cost, fn, kind, key))
                else:
                    fillq.append((cost, fn, kind, key))

            # static fillers: chunk c's V projections + next chunk's Q/K,
            # in fine-grained self-contained units, ordered so deadlines
            # (QK before the chunk, V before pv) hold.
            def push_v(tb):
                for half in range(2):
                    push(433, (lambda tb=tb, h=half: v_half(tb, h)), "v", tb)

            def push_qk(w_sb, dst, bcol, j, cn):
                for half in range(2):
                    push(853, (lambda h=half: qk_half(w_sb, dst, bcol, j,
                                                      cn, h)), "qk", cn)

            for c in range(QC):
                push_v(4 * c)
                if c + 1 < QC:
                    push_qk(wq_sb, qt_sb, 0, 0, c + 1)
                push_v(4 * c + 1)
                if c + 1 < QC:
                    push_qk(wk_sb, kt_sb, 2, 0, c + 1)
                push_v(4 * c + 2)
                if c + 1 < QC:
                    push_qk(wq_sb, qt_sb, 1, 1, c + 1)
                push_v(4 * c + 3)
                if c + 1 < QC:
                    push_qk(wk_sb, kt_sb, 3, 1, c + 1)

            def pv_bundle_units(qb):
                units = []
                for h in range(NH):
                    units.append(((qb + 1) * 27, lambda qb=qb, h=h:
                                  pv_head(qb, h), "pv", qb))

                def snap(qb=qb):
                    y_t[qb] = (y_cur.pop((qb, 0)), y_cur.pop((qb, 1)))
                    transpose_j(qb, 0)
                units.append((53, snap, "t", qb))
                units.append((53, (lambda qb=qb: transpose_j(qb, 1)), "t", qb))
                for e in range(2):
                    units.append((426, (lambda tb=qb, e=e: op_half(tb, e)),
                                  "op", qb))
                return units

            done_fill = 0.0

            exp_end = []      # per s-unit: estimated exp completion
            est_pe = 0.0
            est_act = 0.0

            def drain_kind(kind, key):
                """Force-emit queued fillers of `kind` with key <= key."""
                nonlocal done_fill, est_pe
                rest = []
                for item in fillq:
                    if item[2] == kind and item[3] <= key:
                        item[1]()
                        done_fill += item[0]
                        est_pe += item[0]
                    else:
                        rest.append(item)
                fillq[:] = rest

            for n, (c, jj, hb) in enumerate(s_units):
                if jj == 0 and hb == 0 and c > 0:
                    drain_kind("qk", c)       # Q/K(c) must precede S(c)
                qq0 = max(0, (jj - 4 * c) * 128)
                s_cost = 2 * (512 - qq0) * PE_CYC
                # a ps_s bank recycles when exp (n-2) completes; spend
                # filler only to cover that stall, hold the rest in
                # reserve (leftovers are exp-independent tail work)
                bank_free = exp_end[n - 2] if n >= 2 else 0.0
                while est_pe < bank_free + 300 and fillq:
                    cost, fn, kind, key = fillq.pop(0)
                    fn()
                    est_pe += cost
                    done_fill += cost
                est_pe = max(est_pe, bank_free) + s_cost
                s_pair(hb, c, jj)
                est_act = max(est_act, est_pe + 150) + \
                    0.833 * 2 * (512 - qq0) + 200
                exp_end.append(est_act)
                # unlock pv for q-block qb once both its head-pairs' exps
                # are emitted plus one unit of lag; splice a static unit
                # between the head-pairs so the PSUM ping-pong (h2 reuses
                # h0's bank after its DVE eviction) never stalls PE.
                # out-proj goes to the BACK of the queue: it depends on
                # nothing downstream, so it is the stall reserve that
                # drains on demand (or at the tail).
                if n >= 1:
                    pc, pjj, phb = s_units[n - 1]
                    if phb == 1 and pjj >= 4 * pc:
                        qb = pjj
                        drain_kind("v", qb)   # V tiles feed pv directly
                        units = pv_bundle_units(qb)
                        op_units = units[-2:]
                        units = units[:-2]

                        def take_splice():
                            for i, item in enumerate(fillq):
                                if item[2] in ("v", "qk", "op"):
                                    return fillq.pop(i)
                            return None
                        # [h0 h1 X h2 h3 Y t0 t1]: X covers h2's bank
                        # reuse (waits h0's DVE eviction), Y covers t1
                        # (waits h3's division)
                        s1, s2 = take_splice(), take_splice()
                        if s2 is not None:
                            units = units[:4] + [s2] + units[4:]
                        if s1 is not None:
                            units = units[:2] + [s1] + units[2:]
                        for u in reversed(units):
                            push(*u[:2], kind=u[2], key=u[3], front=True)
                        for u in op_units:
                            push(*u[:2], kind=u[2], key=u[3])

            # tail: remaining queue, interleaved with the last q-block's
            # reduction so nothing serializes behind the final exps.  The
            # backlogged out-projs alternate between the two PSUM pools
            # (ps_pvt frees up as pv(15) retires) so the PSUM rotation
            # never gates back-to-back out-projs.
            units = pv_bundle_units(TB - 1)[:-2]
            heads, snap_u, t1_u = units[:4], units[4], units[5]
            # fillq "op" entries are op-halves: reconstruct (tb, e) pairs
            op_halves = []
            seen = {}
            for _, _, kind, key in fillq:
                if kind == "op":
                    e = seen.get(key, 0)
                    seen[key] = e + 1
                    op_halves.append((key, e))
            for cost, fn, kind, _ in fillq:
                if kind != "op":
                    fn()
            oi = 0
            for u in heads:
                if oi < len(op_halves):
                    tb, e = op_halves[oi]
                    op_half(tb, e)
                    oi += 1
                u[1]()
            # burn the remaining backlog BEFORE the final transposes so
            # their DVE inputs (the divisions) have drained, alternating
            # PSUM pools so the rotation never gates back-to-back ops
            rest_ops = op_halves[oi:]
            for i, (tb, e) in enumerate(rest_ops):
                # the final two stay on ps_mm so the transposes' ps_pvt
                # slots are not held by a just-issued stage copy
                if i >= len(rest_ops) - 2:
                    pool = ps_mm
                else:
                    pool = ps_pvt if (i // 2) % 2 == 0 else ps_mm
                op_half(tb, e, pool=pool)
            snap_u[1]()
            t1_u[1]()
            op_half(TB - 1, 0, pool=ps_pvt)
            op_half(TB - 1, 1, pool=ps_mm)

    if legalize:
        _split_excess_waits(nc)
    return nc


_NC_CACHE = None


def _get_nc():
    global _NC_CACHE
    if _NC_CACHE is None:
        _NC_CACHE = build_attention_nc()
    return _NC_CACHE


def _prep_core_inputs(x, Wq, bq, Wk, bk, Wv, bv, Wp, b, g):
    cols = slice(DHG * g, DHG * (g + 1))
    wv_aug = np.zeros((C, DVA), np.float32)
    for h in range(NH):
        wv_aug[:, 65 * h:65 * h + 64] = \
            Wv[:, DHG * g + DH * h: DHG * g + DH * (h + 1)]
    kk, qq = np.meshgrid(np.arange(128), np.arange(128), indexing="ij")
    mask = (kk <= qq).astype(np.float32)
    mi = np.concatenate([mask, mask, np.eye(128, dtype=np.float32)], axis=1)
    bias = np.stack(
        [bq[cols][0:128], bq[cols][128:256],
         bk[cols][0:128], bk[cols][128:256],
         bv[cols][0:128], bv[cols][128:256]], axis=1)
    return {
        "xt": np.ascontiguousarray(x[b].T).astype(BF16),
        "wq": np.ascontiguousarray(Wq[:, cols]).astype(BF16),
        "wk": np.ascontiguousarray(Wk[:, cols]).astype(BF16),
        "wv": wv_aug.astype(BF16),
        "wp": np.ascontiguousarray(Wp[cols, :]).astype(BF16),
        "mi": mi.astype(BF16),
        "bias": np.ascontiguousarray(bias).astype(np.float32),
    }


def _run(x, Wq, bq, Wk, bk, Wv, bv, Wp, bp, **run_kwargs):
    from concourse.bass_utils import run_bass_kernel_spmd

    x = np.asarray(x, np.float32)
    args = tuple(np.asarray(a, np.float32) for a in (Wq, bq, Wk, bk, Wv, bv, Wp))
    bp = np.asarray(bp, np.float32)

    nc = _get_nc()
    in_maps = [
        _prep_core_inputs(x, *args, b=core // 4, g=core % 4) for core in range(8)
    ]
    res = run_bass_kernel_spmd(nc, in_maps, core_ids=list(range(8)), **run_kwargs)

    B = x.shape[0]
    out = np.zeros((B, T, C), np.float32)
    for core in range(8):
        out[core // 4] += np.asarray(res.results[core]["out"], np.float32)
    out += bp[None, None, :]
    return out, res


def kernel(x, Wq, bq, Wk, bk, Wv, bv, Wp, bp):
    out, _ = _run(x, Wq, bq, Wk, bk, Wv, bv, Wp, bp)
    return out


if __name__ == "__main__":
    rng = np.random.default_rng(0)
    ins = {
        "x": rng.standard_normal((2, T, C), dtype=np.float32),
        "Wq": rng.standard_normal((C, C), dtype=np.float32) * 0.02,
        "bq": rng.standard_normal(C).astype(np.float32) * 0.02,
        "Wk": rng.standard_normal((C, C), dtype=np.float32) * 0.02,
        "bk": rng.standard_normal(C).astype(np.float32) * 0.02,
        "Wv": rng.standard_normal((C, C), dtype=np.float32) * 0.02,
        "bv": rng.standard_normal(C).astype(np.float32) * 0.02,
        "Wp": rng.standard_normal((C, C), dtype=np.float32) * 0.02,
        "bp": rng.standard_normal(C).astype(np.float32) * 0.02,
    }
    got = kernel(**ins)
    print("kernel ran, output shape", got.shape)



# revision 11
# speedup vs baseline: 1.0233x; 1.0233x over previous
"""Causal self-attention on 8 Trainium2 NeuronCores.

Sharding: core c = 4*b + g handles batch b (of 2) and head-group g (4 of 16
heads).  Weights are column-sliced (Wq/Wk/Wv) / row-sliced (Wp) per head
group; each core returns a partial output [T, C] in bf16 that the host
upcasts, sums per batch, and biases.

Per-core dataflow (fp32 PSUM accumulation everywhere):
  Q/K/V projections run in fp8e4m3 DoubleRow perf mode (0.5 cycles/col,
    K=256 per matmul).  The host splits x (scaled x4) and each W (scaled
    x64) into fp8 hi + lo residual pairs; the device accumulates the three
    compensation terms x_hi@W_hi + x_lo@W_hi + x_hi@W_lo in one PSUM group
    (error ~0.2%, 2nd order in fp8 eps), and the 1/256 rescale + per-d bias
    fold into the PSUM->SBUF eviction (DVE two-op tensor_scalar).
  x^T loaded as fp8 hi/lo [128, kt, t] chunk tiles.
  Q^T/K^T = W^T-slice @ x^T -> [128 d, 512 t] per (j, chunk).
  V = x @ Wv_aug -> [t 128, 260] per t-block (aug: 65 cols/head, every 65th
    col is reset to 1.0 by a strided memset -> P@V also yields the softmax
    denominator in column 64 of each head's output).
  S^T[k,q] = K^T.T @ Q^T per (head-pair, k-block): two 64-contraction
    matmuls into one [128, 2, 512] PSUM tile, one merged exp (ACT) for both
    heads, diagonal blocks masked by a Pool multiply.
  PV reoriented [q, d]: y[q, 0:65] += P^T-slice.T @ V-slice per k-block
    (K=128, M=128, N=65 -- full PE array vs the [d, q] orientation's K=64).
    Softmax division is a per-partition reciprocal + tensor_scalar.
  y tiles are PE-transposed back to [d, q] (bv folded into the PSUM->SBUF
    eviction of the transpose), then out_partial = y^T.T @ Wp staged bf16
    and DMA'd per t-block.
"""

import re
import sys

sys.path.insert(0, "/opt/trn_rl_repo")

import numpy as np
import ml_dtypes

import bass_rust
import concourse.bass as bass
import concourse.mybir as mybir
from concourse.tile import TileContext
from concourse.vector_clock import ScopedClock

BF16 = ml_dtypes.bfloat16
FP8 = ml_dtypes.float8_e4m3fn

T = 2048          # sequence length per batch
C = 1024          # model dim
DHG = 256         # head dims per core (4 heads x 64)
DH = 64           # head dim
NH = 4            # heads per core
DVA = NH * (DH + 1)  # 260: V augmented with a ones-column per head
KT = C // 128     # 8 full contraction tiles
KP = KT // 2      # 4 DoubleRow ktile-pairs
TB = T // 128     # 16 row tiles
QC = T // 512     # 4 query chunks
SCALE = 0.125     # 1/sqrt(64)
XS = 4.0          # host prescale of x before fp8 split
WS = 64.0         # host prescale of W before fp8 split
INV = 1.0 / (XS * WS)   # projection eviction rescale


class SplitDrainTileContext(TileContext):
    """Walrus TRN2 codegen rejects >4 sync waits on one instruction; the
    stock TileContext exit drain carries one wait per live proc.  Split them
    into single-wait drains chained on the sync sequencer."""

    def _drain_and_barrier(self, tick_clock, wait_clock):
        gc = tick_clock.global_clock
        ticks = [int(x) for x in re.findall(r"\d+", repr(gc))]
        for proc, tick in [(i, t) for i, t in enumerate(ticks) if t > 0]:
            sub = bass_rust.VectorClock()
            sub.require_at_least(proc, tick)
            inst = self.nc.sync.drain()
            wait_clock.add_sem_waits(inst.ins, ScopedClock({None: sub}))
        self.nc.sync.drain()
        self.nc.all_engine_barrier()
        assert self.sems is not None
        popped = self.nc._tile_sem_poison_stack.pop()
        assert popped is self._sem_poison
        self.nc.clear_and_free_semaphores(list(self.sems.allocated().values()))
        self.nc.all_engine_barrier()


def _split_excess_waits(nc, max_waits=1):
    """Walrus TRN2 codegen allows only ~2 sync waits per instruction.
    Hoist any excess onto same-engine InstNoOp carriers placed immediately
    before the instruction — the engine is in-order, so semantics are
    identical."""
    ctr = 0
    for fn in nc.m.functions:
        for bb in fn.blocks:
            new = []
            for inst in bb.instructions:
                si = inst.sync_info
                if (si and si.on_wait and len(si.on_wait) > max_waits
                        and "Unassigned" not in str(inst.engine)):
                    waits = list(si.on_wait)
                    for w in waits[:-max_waits]:
                        ctr += 1
                        nop = bass_rust.InstNoOp(
                            name=f"wsplit-{ctr}", ins=[], outs=[])
                        nop.engine = inst.engine
                        nop.sync_info = bass_rust.SyncInfo(
                            on_wait=[w], on_update=[])
                        new.append(nop)
                    si.on_wait = waits[-max_waits:]
                new.append(inst)
            bb.instructions = new


def build_attention_nc(legalize=True):
    nc = bass.Bass(num_devices=8)
    dt = mybir.dt

    # fp8 hi/lo compensation pairs (host-split, host-prescaled)
    xt_d = [nc.dram_tensor(f"xt_{v}", [C, T], dt.float8e4,
                           kind="ExternalInput") for v in "hl"]
    wq_d = [nc.dram_tensor(f"wq_{v}", [C, DHG], dt.float8e4,
                           kind="ExternalInput") for v in "hl"]
    wk_d = [nc.dram_tensor(f"wk_{v}", [C, DHG], dt.float8e4,
                           kind="ExternalInput") for v in "hl"]
    wv_d = [nc.dram_tensor(f"wv_{v}", [C, DVA], dt.float8e4,
                           kind="ExternalInput") for v in "hl"]
    wp = nc.dram_tensor("wp", [DHG, C], dt.bfloat16, kind="ExternalInput")
    # mask (x2, for the two heads of a pair) | 128x128 identity
    mi = nc.dram_tensor("mi", [128, 3 * 128], dt.bfloat16, kind="ExternalInput")
    # per-partition bias columns: bq(j0) bq(j1) bk(j0) bk(j1) bv(j0) bv(j1)
    bias = nc.dram_tensor("bias", [128, 6], dt.float32, kind="ExternalInput")
    out = nc.dram_tensor("out", [T, C], dt.bfloat16, kind="ExternalOutput")

    xt_r = [x.rearrange("(k p) t -> p k t", p=128) for x in xt_d]
    wq_r = [w.rearrange("(k p) d -> p k d", p=128) for w in wq_d]
    wk_r = [w.rearrange("(k p) d -> p k d", p=128) for w in wk_d]
    wv_r = [w.rearrange("(k p) d -> p k d", p=128) for w in wv_d]

    with SplitDrainTileContext(nc) as tc:
        with (
            tc.tile_pool(name="weights", bufs=1) as wpool,
            tc.tile_pool(name="acts", bufs=1) as apool,
            tc.tile_pool(name="ptiles", bufs=38) as ppool,
            tc.tile_pool(name="small", bufs=4) as spool,
            tc.tile_pool(name="ostage", bufs=3) as opool,
            tc.tile_pool(name="ps_mm", bufs=2, space="PSUM") as ps_mm,
            tc.tile_pool(name="ps_s", bufs=2, space="PSUM") as ps_s,
            tc.tile_pool(name="ps_pvt", bufs=2, space="PSUM") as ps_pvt,
        ):
            # ---- input loads: wq + xt chunk 0 first (halved so the first
            # Q chain starts early), then K/V weights, the remaining x^T
            # chunks, and Wp.  v indexes the fp8 hi/lo pair. ---------------
            mi_sb = wpool.tile([128, 3, 128], dt.bfloat16)
            bias_sb = wpool.tile([128, 6], dt.float32)
            wq_sb = [[wpool.tile([128, 4, DHG], dt.float8e4, name=f"wq{v}{h}")
                      for h in range(2)] for v in range(2)]
            wk_sb = [[wpool.tile([128, 4, DHG], dt.float8e4, name=f"wk{v}{h}")
                      for h in range(2)] for v in range(2)]
            wv_sb = [[wpool.tile([128, 4, DVA], dt.float8e4, name=f"wv{v}{h}")
                      for h in range(2)] for v in range(2)]
            xt_c0 = [[apool.tile([128, 4, 512], dt.float8e4,
                                 name=f"xtc0_{v}{h}") for h in range(2)]
                     for v in range(2)]
            xt_c = [[apool.tile([128, KT, 512], dt.float8e4,
                                name=f"xtc{v}{cc}") for cc in range(1, QC)]
                    for v in range(2)]
            wp_sb = wpool.tile([128, 2, C], dt.bfloat16)

            for v in range(2):
                nc.sync.dma_start(xt_c0[v][0][:], xt_r[v][:, 0:4, 0:512])
                nc.sync.dma_start(wq_sb[v][0][:], wq_r[v][:, 0:4, :])
                nc.sync.dma_start(xt_c0[v][1][:], xt_r[v][:, 4:8, 0:512])
                nc.sync.dma_start(wq_sb[v][1][:], wq_r[v][:, 4:8, :])
            nc.sync.dma_start(mi_sb[:], mi.rearrange("p (g f) -> p g f", f=128))
            nc.sync.dma_start(bias_sb[:], bias[:])
            for v in range(2):
                nc.sync.dma_start(wk_sb[v][0][:], wk_r[v][:, 0:4, :])
                nc.sync.dma_start(wk_sb[v][1][:], wk_r[v][:, 4:8, :])
                nc.sync.dma_start(wv_sb[v][0][:], wv_r[v][:, 0:4, :])
                nc.sync.dma_start(wv_sb[v][1][:], wv_r[v][:, 4:8, :])
            for cc in range(1, QC):
                for v in range(2):
                    nc.sync.dma_start(
                        xt_c[v][cc - 1][:],
                        xt_r[v][:, :, cc * 512:(cc + 1) * 512])
            nc.sync.dma_start(wp_sb[:], wp.rearrange("(k p) d -> p k d", p=128))

            def xt_pair(v, c, kp):
                """[128, 2, 512] ktile-pair slice of x^T variant v."""
                if c == 0:
                    h, i = kp // 2, kp % 2
                    return xt_c0[v][h][:, 2 * i:2 * i + 2, :]
                return xt_c[v][c - 1][:, 2 * kp:2 * kp + 2, :]

            def w_pair(w_sb, v, kp, lo, hi):
                h, i = kp // 2, kp % 2
                return w_sb[v][h][:, 2 * i:2 * i + 2, lo:hi]

            # (x variant, w variant) compensation terms: hi@hi + lo@hi + hi@lo
            TERMS = ((0, 0), (1, 0), (0, 1))
            DR = mybir.MatmulPerfMode.DoubleRow

            # SBUF activation tiles
            qt_sb = [[apool.tile([128, 512], dt.bfloat16, name=f"qt{j}_{c}")
                      for c in range(QC)] for j in range(2)]
            kt_sb = [[apool.tile([128, 512], dt.bfloat16, name=f"kt{j}_{c}")
                      for c in range(QC)] for j in range(2)]
            v_sb = [apool.tile([128, DVA], dt.bfloat16, name=f"v{tb}")
                    for tb in range(TB)]
            yt_sb = [[apool.tile([128, 512], dt.bfloat16, name=f"yt{j}_{c}")
                      for c in range(QC)] for j in range(2)]

            # ---- unit emitters ------------------------------------------
            def qk_chain(w_sb, dst, bcol, j, c):
                ps = ps_mm.tile([128, 512], dt.float32, tag="mm")
                for n, (xv, wv_) in enumerate(TERMS):
                    for kp in range(KP):
                        nc.tensor.matmul(
                            ps[:],
                            w_pair(w_sb, wv_, kp, j * 128, (j + 1) * 128),
                            xt_pair(xv, c, kp),
                            start=(n == 0 and kp == 0),
                            stop=(n == 2 and kp == KP - 1),
                            perf_mode=DR,
                        )
                nc.vector.tensor_scalar(
                    dst[j][c][:], ps[:], INV, bias_sb[:, bcol:bcol + 1],
                    mybir.AluOpType.mult, mybir.AluOpType.add)

            def qk_half(w_sb, dst, bcol, j, c, half):
                """Self-contained half-width projection: full accumulation
                over 256 of the 512 t-columns + eviction, so arbitrary units
                may interleave without breaking a group."""
                ps = ps_mm.tile([128, 256], dt.float32, tag="mm",
                                name=f"qk{bcol}_{c}_{half}")
                t0 = half * 256
                for n, (xv, wv_) in enumerate(TERMS):
                    for kp in range(KP):
                        nc.tensor.matmul(
                            ps[:],
                            w_pair(w_sb, wv_, kp, j * 128, (j + 1) * 128),
                            xt_pair(xv, c, kp)[:, :, t0:t0 + 256],
                            start=(n == 0 and kp == 0),
                            stop=(n == 2 and kp == KP - 1),
                            perf_mode=DR,
                        )
                nc.vector.tensor_scalar(
                    dst[j][c][:, t0:t0 + 256], ps[:], INV,
                    bias_sb[:, bcol:bcol + 1],
                    mybir.AluOpType.mult, mybir.AluOpType.add)

            def v_half(tb, half):
                ps = ps_mm.tile([128, 130], dt.float32, tag="mm",
                                name=f"v{tb}_{half}")
                d0 = half * 130
                tsl = slice((tb % 4) * 128, (tb % 4 + 1) * 128)
                for n, (xv, wv_) in enumerate(TERMS):
                    for kp in range(KP):
                        nc.tensor.matmul(
                            ps[:],
                            xt_pair(xv, tb // 4, kp)[:, :, tsl],
                            w_pair(wv_sb, wv_, kp, d0, d0 + 130),
                            start=(n == 0 and kp == 0),
                            stop=(n == 2 and kp == KP - 1),
                            perf_mode=DR,
                        )
                nc.vector.tensor_scalar_mul(v_sb[tb][:, d0:d0 + 130], ps[:],
                                            INV)
                # ones-columns for the softmax denominators
                nc.vector.memset(v_sb[tb][:, d0 + DH:d0 + 130:DH + 1], 1.0)

            p_tiles = {}   # (hb, c) -> list over jj of [128, 2, 512] tiles

            def s_pair(hb, c, jj):
                qq0 = max(0, (jj - 4 * c) * 128)
                pss = ps_s.tile([128, 2, 512], dt.float32, tag="s")
                for he in range(2):
                    hp = he * 64
                    nc.tensor.matmul(
                        pss[:, he, qq0:512],
                        kt_sb[hb][jj // 4][hp:hp + 64,
                                           (jj % 4) * 128:(jj % 4 + 1) * 128],
                        qt_sb[hb][c][hp:hp + 64, qq0:512],
                        start=True, stop=True,
                    )
                p = ppool.tile([128, 2, 512], dt.bfloat16, tag="p")
                nc.scalar.activation(
                    p[:, :, qq0:512], pss[:, :, qq0:512],
                    mybir.ActivationFunctionType.Exp, scale=SCALE,
                )
                if jj >= 4 * c:
                    nc.gpsimd.tensor_tensor(
                        p[:, :, qq0:qq0 + 128], p[:, :, qq0:qq0 + 128],
                        mi_sb[:, 0:2, :], mybir.AluOpType.mult,
                    )
                p_tiles[(hb, c)].append(p)

            y_cur = {}     # j -> y_sb tile for the q-block being reduced

            def pv_head(qb, h):
                c, qo = qb // 4, qb % 4
                hb, he = h // 2, h % 2
                ps = ps_pvt.tile([128, 512], dt.float32, tag="pvt",
                                 name=f"pv{qb}_{h}")
                for jj in range(qb + 1):
                    nc.tensor.matmul(
                        ps[:, 0:DH + 1],
                        p_tiles[(hb, c)][jj][:, he, qo * 128:(qo + 1) * 128],
                        v_sb[jj][:, h * 65:h * 65 + 65],
                        start=(jj == 0), stop=(jj == qb),
                    )
                r = spool.tile([128, 1], dt.float32, tag="r", name=f"r{qb}_{h}")
                nc.vector.reciprocal(r[:], ps[:, DH:DH + 1])
                if he == 0:
                    y_cur[(qb, hb)] = spool.tile(
                        [128, 128], dt.bfloat16, tag="y", bufs=8,
                        name=f"y{qb}_{hb}")
                nc.vector.tensor_scalar_mul(
                    y_cur[(qb, hb)][:, he * 64:(he + 1) * 64], ps[:, 0:DH], r[:])

            y_t = {}       # qb -> (y_sb j0, y_sb j1) awaiting transpose

            def transpose_j(qb, j):
                c, qo = qb // 4, qb % 4
                pst = ps_pvt.tile([128, 128], dt.bfloat16, tag="pvt",
                                  name=f"tp{qb}_{j}")
                nc.tensor.transpose(pst[:], y_t[qb][j][:], mi_sb[:, 2, :])
                nc.vector.tensor_scalar_add(
                    yt_sb[j][c][:, qo * 128:(qo + 1) * 128], pst[:],
                    bias_sb[:, 4 + j:5 + j])

            def op_half(tb, e, pool=None):
                pool = pool or ps_mm
                ps = pool.tile([128, 512], dt.float32, tag=pool.name[3:],
                               name=f"op{tb}_{e}")
                for j in range(2):
                    nc.tensor.matmul(
                        ps[:],
                        yt_sb[j][tb // 4][:, (tb % 4) * 128:(tb % 4 + 1) * 128],
                        wp_sb[:, j, e * 512:(e + 1) * 512],
                        start=(j == 0), stop=(j == 1),
                    )
                st = opool.tile([128, 512], dt.bfloat16, tag="st",
                                name=f"st{tb}_{e}")
                # early-draining out-projs overlap the mid-kernel where ACT
                # has slack; keep the endgame (tb>=10) copies off ACT, which
                # is saturated by the last chunk's exps there
                if e == 1 and tb < 10:
                    nc.scalar.copy(st[:], ps[:])
                else:
                    nc.vector.tensor_copy(st[:], ps[:])
                nc.sync.dma_start(
                    out[tb * 128:(tb + 1) * 128, e * 512:(e + 1) * 512],
                    st[:])

            # ---- emission schedule --------------------------------------
            # PE warmup: ~3us of dummy matmuls ramps the PE p-state to full
            # speed while the first DMAs are still in flight.
            warm_sb = wpool.tile([128, 128], dt.bfloat16)
            nc.vector.memset(warm_sb[:], 0.0)
            psw = ps_mm.tile([128, 128], dt.float32, tag="mm", name="warm")
            NWARM = 50
            for i in range(NWARM):
                nc.tensor.matmul(psw[:], warm_sb[:], warm_sb[:],
                                 start=(i == 0), stop=(i == NWARM - 1))

            # chunk 0's Q/K projections
            qk_chain(wq_sb, qt_sb, 0, 0, 0)
            qk_chain(wq_sb, qt_sb, 1, 1, 0)
            qk_chain(wk_sb, kt_sb, 2, 0, 0)
            qk_chain(wk_sb, kt_sb, 3, 1, 0)
            for hb in range(2):
                for c in range(QC):
                    p_tiles[(hb, c)] = []

            # Global flattened S stream: all (c, jj, hb) in order, zipped
            # with a single filler queue drained proportionally (plus
            # stall-covering) so PE stays fed and ACT (exp) never gates a
            # bunched region.  PE-time estimates pace the zip; correctness
            # only relies on emission order, the tile deps do the rest.
            s_units = [(c, jj, hb)
                       for c in range(QC)
                       for jj in range(4 * c + 4)
                       for hb in range(2)]
            NS = len(s_units)

            PE_CYC = 0.4166
            fillq = []        # (cost_ns, closure, kind, key)

            def push(cost, fn, kind=None, key=None, front=False):
                if front:
                    fillq.insert(0, (cost, fn, kind, key))
                else:
                    fillq.append((cost, fn, kind, key))

            # static fillers: chunk c's V projections + next chunk's Q/K,
            # in fine-grained self-contained units, ordered so deadlines
            # (QK before the chunk, V before pv) hold.
            def push_v(tb):
                for half in range(2):
                    push(325, (lambda tb=tb, h=half: v_half(tb, h)), "v", tb)

            def push_qk(w_sb, dst, bcol, j, cn):
                for half in range(2):
                    push(640, (lambda h=half: qk_half(w_sb, dst, bcol, j,
                                                      cn, h)), "qk", cn)

            for c in range(QC):
                push_v(4 * c)
                if c + 1 < QC:
                    push_qk(wq_sb, qt_sb, 0, 0, c + 1)
                push_v(4 * c + 1)
                if c + 1 < QC:
                    push_qk(wk_sb, kt_sb, 2, 0, c + 1)
                push_v(4 * c + 2)
                if c + 1 < QC:
                    push_qk(wq_sb, qt_sb, 1, 1, c + 1)
                push_v(4 * c + 3)
                if c + 1 < QC:
                    push_qk(wk_sb, kt_sb, 3, 1, c + 1)

            def pv_bundle_units(qb):
                units = []
                for h in range(NH):
                    units.append(((qb + 1) * 27, lambda qb=qb, h=h:
                                  pv_head(qb, h), "pv", qb))

                def snap(qb=qb):
                    y_t[qb] = (y_cur.pop((qb, 0)), y_cur.pop((qb, 1)))
                    transpose_j(qb, 0)
                units.append((53, snap, "t", qb))
                units.append((53, (lambda qb=qb: transpose_j(qb, 1)), "t", qb))
                for e in range(2):
                    units.append((426, (lambda tb=qb, e=e: op_half(tb, e)),
                                  "op", qb))
                return units

            done_fill = 0.0

            exp_end = []      # per s-unit: estimated exp completion
            est_pe = 0.0
            est_act = 0.0

            def drain_kind(kind, key):
                """Force-emit queued fillers of `kind` with key <= key."""
                nonlocal done_fill, est_pe
                rest = []
                for item in fillq:
                    if item[2] == kind and item[3] <= key:
                        item[1]()
                        done_fill += item[0]
                        est_pe += item[0]
                    else:
                        rest.append(item)
                fillq[:] = rest

            for n, (c, jj, hb) in enumerate(s_units):
                if jj == 0 and hb == 0 and c > 0:
                    drain_kind("qk", c)       # Q/K(c) must precede S(c)
                qq0 = max(0, (jj - 4 * c) * 128)
                s_cost = 2 * (512 - qq0) * PE_CYC
                # a ps_s bank recycles when exp (n-2) completes; spend
                # filler only to cover that stall, hold the rest in
                # reserve (leftovers are exp-independent tail work)
                bank_free = exp_end[n - 2] if n >= 2 else 0.0
                while est_pe < bank_free + 300 and fillq:
                    cost, fn, kind, key = fillq.pop(0)
                    fn()
                    est_pe += cost
                    done_fill += cost
                est_pe = max(est_pe, bank_free) + s_cost
                s_pair(hb, c, jj)
                est_act = max(est_act, est_pe + 150) + \
                    0.833 * 2 * (512 - qq0) + 200
                exp_end.append(est_act)
                # unlock pv for q-block qb once both its head-pairs' exps
                # are emitted plus one unit of lag; splice a static unit
                # between the head-pairs so the PSUM ping-pong (h2 reuses
                # h0's bank after its DVE eviction) never stalls PE.
                # out-proj goes to the BACK of the queue: it depends on
                # nothing downstream, so it is the stall reserve that
                # drains on demand (or at the tail).
                if n >= 1:
                    pc, pjj, phb = s_units[n - 1]
                    if phb == 1 and pjj >= 4 * pc:
                        qb = pjj
                        drain_kind("v", qb)   # V tiles feed pv directly
                        units = pv_bundle_units(qb)
                        op_units = units[-2:]
                        units = units[:-2]

                        def take_splice():
                            for i, item in enumerate(fillq):
                                if item[2] in ("v", "qk", "op"):
                                    return fillq.pop(i)
                            return None
                        # [h0 h1 X h2 h3 Y t0 t1]: X covers h2's bank
                        # reuse (waits h0's DVE eviction), Y covers t1
                        # (waits h3's division)
                        s1, s2 = take_splice(), take_splice()
                        if s2 is not None:
                            units = units[:4] + [s2] + units[4:]
                        if s1 is not None:
                            units = units[:2] + [s1] + units[2:]
                        for u in reversed(units):
                            push(*u[:2], kind=u[2], key=u[3], front=True)
                        for u in op_units:
                            push(*u[:2], kind=u[2], key=u[3])

            # tail: remaining queue, interleaved with the last q-block's
            # reduction so nothing serializes behind the final exps.  The
            # backlogged out-projs alternate between the two PSUM pools
            # (ps_pvt frees up as pv(15) retires) so the PSUM rotation
            # never gates back-to-back out-projs.
            units = pv_bundle_units(TB - 1)[:-2]
            heads, snap_u, t1_u = units[:4], units[4], units[5]
            # fillq "op" entries are op-halves: reconstruct (tb, e) pairs
            op_halves = []
            seen = {}
            for _, _, kind, key in fillq:
                if kind == "op":
                    e = seen.get(key, 0)
                    seen[key] = e + 1
                    op_halves.append((key, e))
            for cost, fn, kind, _ in fillq:
                if kind != "op":
                    fn()
            oi = 0
            for u in heads:
                if oi < len(op_halves):
                    tb, e = op_halves[oi]
                    op_half(tb, e)
                    oi += 1
                u[1]()
            # burn the remaining backlog BEFORE the final transposes so
            # their DVE inputs (the divisions) have drained, alternating
            # PSUM pools so the rotation never gates back-to-back ops
            rest_ops = op_halves[oi:]
            for i, (tb, e) in enumerate(rest_ops):
                # the final two stay on ps_mm so the transposes' ps_pvt
                # slots are not held by a just-issued stage copy
                if i >= len(rest_ops) - 2:
                    pool = ps_mm
                else:
                    pool = ps_pvt if (i // 2) % 2 == 0 else ps_mm
                op_half(tb, e, pool=pool)
            snap_u[1]()
            t1_u[1]()
            op_half(TB - 1, 0, pool=ps_pvt)
            op_half(TB - 1, 1, pool=ps_mm)

    if legalize:
        _split_excess_waits(nc)
    return nc


_NC_CACHE = None


def _get_nc():
    global _NC_CACHE
    if _NC_CACHE is None:
        _NC_CACHE = build_attention_nc()
    return _NC_CACHE


def _fp8_split(a):
    """hi = fp8(a), lo = fp8(a - hi); hi + lo represents a to ~0.1%."""
    hi = np.ascontiguousarray(a).astype(FP8)
    lo = (a - hi.astype(np.float32)).astype(FP8)
    return hi, lo


def _prep_core_inputs(x, Wq, bq, Wk, bk, Wv, bv, Wp, b, g):
    cols = slice(DHG * g, DHG * (g + 1))
    wv_aug = np.zeros((C, DVA), np.float32)
    for h in range(NH):
        wv_aug[:, 65 * h:65 * h + 64] = \
            Wv[:, DHG * g + DH * h: DHG * g + DH * (h + 1)]
    kk, qq = np.meshgrid(np.arange(128), np.arange(128), indexing="ij")
    mask = (kk <= qq).astype(np.float32)
    mi = np.concatenate([mask, mask, np.eye(128, dtype=np.float32)], axis=1)
    bias = np.stack(
        [bq[cols][0:128], bq[cols][128:256],
         bk[cols][0:128], bk[cols][128:256],
         bv[cols][0:128], bv[cols][128:256]], axis=1)
    xt_h, xt_l = _fp8_split(XS * x[b].T)
    wq_h, wq_l = _fp8_split(WS * Wq[:, cols])
    wk_h, wk_l = _fp8_split(WS * Wk[:, cols])
    wv_h, wv_l = _fp8_split(WS * wv_aug)
    return {
        "xt_h": xt_h, "xt_l": xt_l,
        "wq_h": wq_h, "wq_l": wq_l,
        "wk_h": wk_h, "wk_l": wk_l,
        "wv_h": wv_h, "wv_l": wv_l,
        "wp": np.ascontiguousarray(Wp[cols, :]).astype(BF16),
        "mi": mi.astype(BF16),
        "bias": np.ascontiguousarray(bias).astype(np.float32),
    }


def _run(x, Wq, bq, Wk, bk, Wv, bv, Wp, bp, **run_kwargs):
    from concourse.bass_utils import run_bass_kernel_spmd

    x = np.asarray(x, np.float32)
    args = tuple(np.asarray(a, np.float32) for a in (Wq, bq, Wk, bk, Wv, bv, Wp))
    bp = np.asarray(bp, np.float32)

    nc = _get_nc()
    in_maps = [
        _prep_core_inputs(x, *args, b=core // 4, g=core % 4) for core in range(8)
    ]
    res = run_bass_kernel_spmd(nc, in_maps, core_ids=list(range(8)), **run_kwargs)

    B = x.shape[0]
    out = np.zeros((B, T, C), np.float32)
    for core in range(8):
        out[core // 4] += np.asarray(res.results[core]["out"], np.float32)
    out += bp[None, None, :]
    return out, res


def kernel(x, Wq, bq, Wk, bk, Wv, bv, Wp, bp):
    out, _ = _run(x, Wq, bq, Wk, bk, Wv, bv, Wp, bp)
    return out


if __name__ == "__main__":
    rng = np.random.default_rng(0)
    ins = {
        "x": rng.standard_normal((2, T, C), dtype=np.float32),
        "Wq": rng.standard_normal((C, C), dtype=np.float32) * 0.02,
        "bq": rng.standard_normal(C).astype(np.float32) * 0.02,
        "Wk": rng.standard_normal((C, C), dtype=np.float32) * 0.02,
        "bk": rng.standard_normal(C).astype(np.float32) * 0.02,
        "Wv": rng.standard_normal((C, C), dtype=np.float32) * 0.02,
        "bv": rng.standard_normal(C).astype(np.float32) * 0.02,
        "Wp": rng.standard_normal((C, C), dtype=np.float32) * 0.02,
        "bp": rng.standard_normal(C).astype(np.float32) * 0.02,
    }
    got = kernel(**ins)
    print("kernel ran, output shape", got.shape)

